# revision 15
# baseline (speedup 1.0000x reference)
"""Causal multi-head attention block (qkv proj -> causal softmax attention ->
out proj) for Trainium2, distributed over 8 NeuronCores.

Sharding: 4-way data parallel over batch x 2-way tensor parallel over heads
(8 heads per core). Each core computes, for its (batch, head-group):
  - QKV projection directly in transposed-per-head layout (Q^T/K^T [hd, T])
  - causal softmax attention entirely in the "scores transposed" [k, q]
    orientation (row-sums via an appended ones-column in the PV matmul)
  - its partial output projection (row-parallel W_proj split)
Host side packs/shards inputs, and sums the two head-group partials per batch.

v2: all matmul operands in bf16 (PSUM accumulation stays fp32):
  - two S^T head matmuls in disjoint PE row-groups run truly concurrent
    (fp32r serializes them - its moving operand needs both rhs XBUSes)
  - no N<256 fp32r rate penalty, so causal bounds are exact 128-granular
  - FWL halves LDWEIGHTS time; DVE/DMA volume halves
Attention is software-pipelined (S^T emitted one kt ahead of PV), and the
next pair's projection chains are woven between attention kts as PE filler
so the tensor engine never idles (keeps the HAM clock-gate at full rate).
The output projection is woven into the last pair's attention the same way.
"""
import numpy as np
import ml_dtypes
from collections import deque
from contextlib import ExitStack

import concourse.bacc as bacc
import concourse.tile as tile
from concourse import mybir
from concourse.alu_op_type import AluOpType
from concourse.bass_utils import run_bass_kernel_spmd

F32 = mybir.dt.float32
BF16 = mybir.dt.bfloat16
import os as _os
# bf16 matmul operands measured ~15% faster but produce sporadic wrong
# results on hardware (PSUM-reader races only seen with bf16 operand paths;
# CoreSim race detector is clean) - default to the stable float32r.
_BF16_OPT_IN = bool(_os.environ.get("KERNEL_BF16"))
MMDT = BF16 if _BF16_OPT_IN else mybir.dt.float32r
QKDT = MMDT
EXP = mybir.ActivationFunctionType.Exp

T, C, HD, HL = 2048, 1024, 64, 8   # seq, d_model, head_dim, heads-per-core
N_CORES = 8
BF = ml_dtypes.bfloat16 if _BF16_OPT_IN else np.float32

_CACHE = {}
last_results = None   # BassKernelResults of the most recent run (for test.py)


def _build_nc():
    import os
    dbg = bool(os.environ.get("KERNEL_DEBUG"))
    nc = bacc.Bacc("TRN2", target_bir_lowering=False, debug=False,
                   num_devices=N_CORES)
    xT = nc.dram_tensor("xT", [C, T], MMDT, kind="ExternalInput").ap()
    wqk = nc.dram_tensor("wqk", [C, 1024], MMDT, kind="ExternalInput").ap()
    wv = nc.dram_tensor("wv", [C, 512], MMDT, kind="ExternalInput").ap()
    wp = nc.dram_tensor("wp", [512, C], BF16, kind="ExternalInput").ap()
    bqk = nc.dram_tensor("bqk", [128, 8], F32, kind="ExternalInput").ap()
    bo = nc.dram_tensor("bo", [128, 8], F32, kind="ExternalInput").ap()
    tri = nc.dram_tensor("trimask", [128, 128], MMDT, kind="ExternalInput").ap()
    vones = nc.dram_tensor("vones", [128, 32], MMDT, kind="ExternalInput").ap()
    outT = nc.dram_tensor("outT", [C, T], BF16, kind="ExternalOutput").ap()
    if dbg:
        qkd = nc.dram_tensor("qkd", [4, 128, 2, T], QKDT, kind="ExternalOutput").ap()
        vtd = nc.dram_tensor("vtd", [4, 128, 2, 16, 65], MMDT, kind="ExternalOutput").ap()
        ytd = nc.dram_tensor("ytd", [4, 128, 4, 512], BF16, kind="ExternalOutput").ap()
        ptd = nc.dram_tensor("ptd", [4, 128, 1024], MMDT, kind="ExternalOutput").ap()
        ysd = nc.dram_tensor("ysd", [2, 65, 512], F32, kind="ExternalOutput").ap()

    with tile.TileContext(nc) as tc, ExitStack() as ctx:
        def pool(name, bufs, space="SBUF"):
            return ctx.enter_context(tc.tile_pool(name=name, bufs=bufs, space=space))

        constp = pool("const", 1)
        xtp = pool("xtp", 1)
        wqkp = pool("wqkp", 2)
        wvp = pool("wvp", 2 if _BF16_OPT_IN else 1)
        wpfp = pool("wpfp", 1)
        ytp = pool("ytp", 1)
        vp = pool("vp", 4)
        qkp = pool("qkp", 2)
        bcp = pool("bcp", 2 if _BF16_OPT_IN else 1)
        ptp = pool("ptp", 3)
        osbp = pool("osbp", 2 if _BF16_OPT_IN else 1)
        rpp = pool("rpp", 2 if _BF16_OPT_IN else 1)
        pp = pool("pp", 2, "PSUM")      # projection chains: 2 banks
        stp = pool("stp", 2, "PSUM")    # S^T tiles: 2x2 = 4 banks
        yp = pool("yp", 2, "PSUM")      # PV accumulators: 2 banks

        bqk_t = constp.tile([128, 8], F32)
        nc.sync.dma_start(out=bqk_t, in_=bqk)
        # tiny warmup exp so the ACT table-set DMA happens during the input
        # loads instead of stalling the first real softmax tile
        warm = constp.tile([1, 8], F32)
        nc.scalar.activation(warm[0:1, :], bqk_t[0:1, :], EXP)
        wqkr = wqk.rearrange("(c p) m -> p c m", p=128)   # [128, 8, 1024]
        wvr = wv.rearrange("(c p) m -> p c m", p=128)     # [128, 8, 512]
        wpr = wp.rearrange("(c p) m -> p c m", p=128)     # [128, 4, 1024]

        xt = xtp.tile([128, 8, 4, 512], MMDT)
        xTr = xT.rearrange("(c p) (tb n) -> p c tb n", p=128, n=512)

        wqk_tiles = {}
        qk_tiles = {}
        wv_tiles = {}
        v_tiles = {}

        def build_pair(pr):
            """Allocate + start DMA for pair pr's QK weights and qk output."""
            wt = wqkp.tile([128, 8, 256], MMDT, name="wqkpr")
            wqk_tiles[pr] = wt
            nc.sync.dma_start(out=wt,
                              in_=wqkr[:, :, 2 * pr * 128:(2 * pr + 2) * 128])
            qk_tiles[pr] = qkp.tile([128, 2, T], QKDT, name="qk")

        def build_vgroup(g):
            """Allocate + start DMA for group g's V weights and V tiles."""
            wt = wvp.tile([128, 8, 256], MMDT, name="wvg")
            wv_tiles[g] = wt
            nc.sync.dma_start(out=wt, in_=wvr[:, :, g * 256:(g + 1) * 256])
            for prl in (2 * g, 2 * g + 1):
                vt = vp.tile([128, 2, 16, 65], MMDT, name="v")
                v_tiles[prl] = vt
                # rowsum ones-column (col 64 of every k-tile)
                nc.sync.dma_start(
                    out=vt[:, 0:2, :, 64:65],
                    in_=vones.rearrange("p (a kt one) -> p a kt one", a=2, one=1))

        def qk_chain(pr, tb, mi):
            """One Q^T/K^T projection chain: psum rows = head A | head B."""
            def emit():
                wt = wqk_tiles[pr]
                p = pp.tile([128, 512], F32, name="pp")
                for c in range(8):
                    nc.tensor.matmul(p, lhsT=wt[:, c, mi * 128:(mi + 1) * 128],
                                     rhs=xt[:, c, tb, :],
                                     start=(c == 0), stop=(c == 7))
                nc.vector.tensor_scalar_add(
                    out=qk_tiles[pr][:, mi, tb * 512:(tb + 1) * 512], in0=p,
                    scalar1=bqk_t[:, 2 * pr + mi: 2 * pr + mi + 1])
            return emit

        def v_chain(g, tb, tt):
            """V^T projection chain for head pairs 2g,2g+1 (N=256)."""
            def emit():
                p = pp.tile([128, 512], F32, name="pp")
                for c in range(8):
                    nc.tensor.matmul(p[:, 0:256],
                                     lhsT=xt[:, c, tb, tt * 128:(tt + 1) * 128],
                                     rhs=wv_tiles[g][:, c, :],
                                     start=(c == 0), stop=(c == 7))
                kt = tb * 4 + tt
                for j, prl in enumerate((2 * g, 2 * g + 1)):
                    src = p[:, j * 128:(j + 1) * 128].rearrange(
                        "p (a d) -> p a d", a=2)
                    nc.vector.tensor_copy(
                        out=v_tiles[prl][:, 0:2, kt, 0:64], in_=src)
            return emit

        def o_chain(tb, oc):
            """Output projection chain for one [128 out-dims, 512 t] block."""
            def emit():
                p = pp.tile([128, 512], F32, name="pp")
                for yc in range(4):
                    nc.tensor.matmul(p, lhsT=wp_t[:, yc, oc * 128:(oc + 1) * 128],
                                     rhs=yts[tb][:, yc, :],
                                     start=(yc == 0), stop=(yc == 3))
                o = osbp.tile([128, 512], BF16, name="osb")
                nc.vector.tensor_scalar_add(out=o, in0=p,
                                            scalar1=bo_t[:, oc:oc + 1])
                nc.sync.dma_start(
                    out=outT[oc * 128:(oc + 1) * 128, tb * 512:(tb + 1) * 512],
                    in_=o)
            return emit

        def attention(pr, fillers_by_qb, tail=None):
            """Causal attention for pair pr, software-pipelined one kt ahead.

            fillers_by_qb[qb] is a deque of chain closures woven between the
            S^T and PV matmuls so the PE stream stays dense while the ACT
            engine streams the exps.
            """
            qk = qk_tiles[pr]
            vt = v_tiles[pr]
            for qb in range(4):
                K = 4 * qb + 4
                fillers = fillers_by_qb.get(qb, deque())
                ys = [yp.tile([65, 512], F32, name="y") for _ in range(2)]
                sts = {}

                def emit_st(kt):
                    d = kt - 4 * qb
                    lo = 0 if d < 0 else 128 * d
                    st = stp.tile([128, 1024], F32, name="st")
                    sts[kt] = (st, lo)
                    for a in range(2):
                        po = a * 64
                        # S^T[k, q] = K^T_chunk.T @ Q^T ; heads A/B in
                        # disjoint row-groups -> concurrent at bf16
                        nc.tensor.matmul(
                            st[:, a * 512 + lo:(a + 1) * 512],
                            lhsT=qk[po:po + 64, 1, kt * 128:(kt + 1) * 128],
                            rhs=qk[po:po + 64, 0,
                                   qb * 512 + lo:(qb + 1) * 512],
                            start=True, stop=True,
                            tile_position=(po, 0))

                emit_st(0)
                for kt in range(K):
                    if kt + 1 < K:
                        emit_st(kt + 1)
                    st, lo = sts.pop(kt)
                    d = kt - 4 * qb
                    pt = ptp.tile([128, 1024], MMDT, name="pt")
                    if lo == 0:
                        nc.scalar.activation(pt[:, 0:1024], st[:, 0:1024], EXP)
                    else:
                        # strided AP covering exactly the written per-head
                        # ranges [lo:512] and [512+lo:1024]
                        str_o = pt.rearrange("p (a n) -> p a n", a=2)[:, :, lo:512]
                        str_i = st.rearrange("p (a n) -> p a n", a=2)[:, :, lo:512]
                        nc.scalar.activation(str_o, str_i, EXP)
                    if d >= 0:
                        for a in range(2):
                            blk = slice(a * 512 + lo, a * 512 + lo + 128)
                            nc.vector.tensor_tensor(out=pt[:, blk],
                                                    in0=pt[:, blk],
                                                    in1=tri_t,
                                                    op=AluOpType.mult)
                    if dbg and pr == 0 and qb == 0:
                        nc.sync.dma_start(out=ptd[kt], in_=pt)
                    # PE filler between the (already emitted) S^T and the
                    # exp-dependent PV so the in-order PE queue never stalls
                    npop = 2 if len(fillers) > (K - kt) else 1
                    for _ in range(npop):
                        if fillers:
                            fillers.popleft()()
                    for a in range(2):
                        nc.tensor.matmul(
                            ys[a][:, lo:512],
                            lhsT=vt[:, a, kt, :],
                            rhs=pt[:, a * 512 + lo:(a + 1) * 512],
                            start=(kt == 0), stop=(kt == K - 1))
                while fillers:
                    fillers.popleft()()
                if dbg and pr == 0 and qb == 0:
                    for a in range(2):
                        ysb = rpp.tile([65, 512], F32, name=f"ysb{a}")
                        nc.vector.tensor_copy(out=ysb, in_=ys[a])
                        nc.sync.dma_start(out=ysd[a], in_=ysb)
                for a in range(2):
                    rs = rpp.tile([1, 512], F32, name="rs")
                    nc.vector.tensor_copy(out=rs[0:1, :], in_=ys[a][64:65, :])
                    rc = rpp.tile([1, 512], F32, name="rc")
                    nc.vector.reciprocal_approx_fast(out=rc[0:1, :],
                                                     in_=rs[0:1, :])
                    bct = bcp.tile([64, 512], F32, name="bc")
                    nc.gpsimd.partition_broadcast(bct, rc[0:1, :], channels=64)
                    nc.vector.tensor_tensor(
                        out=yts[qb][a * 64:(a + 1) * 64, pr, :],
                        in0=ys[a][0:64, :], in1=bct, op=AluOpType.mult)
            if tail is not None:
                while tail:
                    tail.popleft()()
            if dbg:
                nc.sync.dma_start(out=qkd[pr], in_=qk)
                nc.sync.dma_start(out=vtd[pr], in_=vt)
                if pr == 3:
                    for tb in range(4):
                        nc.sync.dma_start(out=ytd[tb], in_=yts[tb])

        # ---- prologue: pair0 weights interleaved with x tb0, then the rest
        build_pair(0)
        for c in range(8):
            nc.sync.dma_start(out=xt[:, c, 0], in_=xTr[:, c, 0])
        build_vgroup(0)
        bo_t = constp.tile([128, 8], F32)
        nc.sync.dma_start(out=bo_t, in_=bo)
        tri_t = constp.tile([128, 128], MMDT)
        nc.sync.dma_start(out=tri_t, in_=tri)
        for tb in range(1, 4):
            for c in range(8):
                nc.sync.dma_start(out=xt[:, c, tb], in_=xTr[:, c, tb])

        # y^T staging, one tile per t-block: each [128(pair rows), pair, 512]
        yts = [ytp.tile([128, 4, 512], BF16, name=f"yt{tb}") for tb in range(4)]

        # pair0 tb0 projections must precede attention(0)
        for mi in range(2):
            qk_chain(0, 0, mi)()
        for tt in range(4):
            v_chain(0, 0, tt)()

        build_pair(1)
        build_vgroup(1)

        f0 = {}
        for qb in range(3):
            f0[qb] = deque([qk_chain(0, qb + 1, mi) for mi in range(2)]
                           + [v_chain(0, qb + 1, tt) for tt in range(4)])
        f0[2].extend(qk_chain(1, tb, mi) for tb in range(2) for mi in range(2))
        f0[3] = deque([qk_chain(1, tb, mi) for tb in range(2, 4) for mi in range(2)]
                      + [v_chain(1, tb, tt) for tb in range(2) for tt in range(4)])
        attention(0, f0)

        build_pair(2)
        f1 = {0: deque(qk_chain(2, tb, mi) for tb in range(2) for mi in range(2)),
              2: deque(qk_chain(2, tb, mi) for tb in range(2, 4) for mi in range(2))}
        attention(1, f1)

        build_pair(3)
        wp_t = wpfp.tile([128, 4, 1024], BF16, name="wpfull")
        nc.sync.dma_start(out=wp_t, in_=wpr)
        f2 = {0: deque(v_chain(1, 2, tt) for tt in range(4)),
              1: deque(v_chain(1, 3, tt) for tt in range(4)),
              2: deque(qk_chain(3, tb, mi) for tb in range(2) for mi in range(2)),
              3: deque(qk_chain(3, tb, mi) for tb in range(2, 4) for mi in range(2))}
        attention(2, f2)

        f3 = {1: deque(o_chain(0, oc) for oc in range(8)),
              2: deque(o_chain(1, oc) for oc in range(8)),
              3: deque(o_chain(2, oc) for oc in range(8))}
        attention(3, f3, tail=deque(o_chain(3, oc) for oc in range(8)))

    nc.compile()
    return nc


def _host_pack(x, W_attn, b_attn, W_proj, b_proj):
    """Per-core input dicts: shard batch (4-way) x head-group (2-way)."""
    tri = (np.arange(128)[None, :] >= np.arange(128)[:, None]).astype(np.float32)
    in_maps = []
    for i in range(N_CORES):
        b, hg = i // 2, i % 2
        h0 = hg * HL
        xT = np.ascontiguousarray(x[b].T).astype(BF)
        wqk = np.empty((C, 1024), np.float32)
        bqk = np.empty((128, 8), np.float32)
        for pr in range(4):
            hA, hB = h0 + 2 * pr, h0 + 2 * pr + 1
            mq, mk = 2 * pr, 2 * pr + 1
            wqk[:, mq * 128:mq * 128 + 64] = W_attn[:, hA * 64:(hA + 1) * 64] * 0.125
            wqk[:, mq * 128 + 64:(mq + 1) * 128] = W_attn[:, hB * 64:(hB + 1) * 64] * 0.125
            wqk[:, mk * 128:mk * 128 + 64] = W_attn[:, C + hA * 64:C + (hA + 1) * 64]
            wqk[:, mk * 128 + 64:(mk + 1) * 128] = W_attn[:, C + hB * 64:C + (hB + 1) * 64]
            bqk[0:64, mq] = b_attn[hA * 64:(hA + 1) * 64] * 0.125
            bqk[64:128, mq] = b_attn[hB * 64:(hB + 1) * 64] * 0.125
            bqk[0:64, mk] = b_attn[C + hA * 64:C + (hA + 1) * 64]
            bqk[64:128, mk] = b_attn[C + hB * 64:C + (hB + 1) * 64]
        wv = np.ascontiguousarray(W_attn[:, 2 * C + h0 * 64:2 * C + (h0 + HL) * 64])
        wp = np.ascontiguousarray(W_proj[h0 * 64:(h0 + HL) * 64, :])
        bv = b_attn[2 * C + h0 * 64:2 * C + (h0 + HL) * 64]
        combined = (bv.astype(np.float64) @ wp.astype(np.float64))
        if hg == 0:
            combined = combined + b_proj
        bo = np.ascontiguousarray(combined.astype(np.float32).reshape(8, 128).T)
        in_maps.append(dict(xT=xT, wqk=wqk.astype(BF), bqk=bqk,
                            wv=wv.astype(BF), wp=wp.astype(ml_dtypes.bfloat16), bo=bo,
                            trimask=tri.astype(BF),
                            vones=np.ones((128, 32), BF)))
    return in_maps


def kernel(x, W_attn, b_attn, W_proj, b_proj):
    global last_results
    import os
    x = np.ascontiguousarray(np.asarray(x, dtype=np.float32))
    W_attn = np.asarray(W_attn, dtype=np.float32)
    b_attn = np.asarray(b_attn, dtype=np.float32)
    W_proj = np.asarray(W_proj, dtype=np.float32)
    b_proj = np.asarray(b_proj, dtype=np.float32)

    if "nc" not in _CACHE:
        _CACHE["nc"] = _build_nc()
    nc = _CACHE["nc"]

    in_maps = _host_pack(x, W_attn, b_attn, W_proj, b_proj)
    trace = bool(os.environ.get("KERNEL_TRACE"))
    res = run_bass_kernel_spmd(nc, in_maps, core_ids=list(range(N_CORES)),
                               trace=trace)
    last_results = res

    B = x.shape[0]
    out = np.empty((B, T, C), np.float32)
    for b in range(B):
        out[b] = (res.results[2 * b]["outT"].astype(np.float32)
                  + res.results[2 * b + 1]["outT"].astype(np.float32)).T
    return out
